# revision 1
# baseline (speedup 1.0000x reference)
"""ConvLSTM latent-cell kernel for 8x Trainium2 NeuronCores.

Model (matches reference):
  x = x_audio + pe(midi_notes)                       [B,T,16,64,64]
  layer0 = bidirectional ConvLSTM(16 -> 32), 3x3 SAME
  layer1 = bidirectional ConvLSTM(64 -> 32), 3x3 SAME
  out    = conv3x3(layer1_out[T-1], 64 -> 64) + bias [B,64,64,64]

Sharding: data-parallel over batch, B=16 -> 2 images per core, weights
replicated, no collectives.  Only h[-1] of layer 1 is consumed, so the
layer-1 backward scan runs a single step (its t=T-1 output is step 0 of
the reversed scan).

Per-core program: channels on the matmul contraction dim; conv = 9
shifted matmuls (full M=128 output channels) accumulating in PSUM from
a zero-padded [Cin, 66*66] bf16 input buffer.  Gate nonlinearities run
on ScalarE while evacuating PSUM; gate tensors are then re-tiled via
SBUF->SBUF DMAs into a "folded" [128, 1024] layout (4 spatial blocks x
32 gate channels) so the LSTM cell elementwise math uses all 128
partitions at base partition 0 (VectorE requires equal input base
partitions).  Cell state and gate math stay fp32; bf16 is used only for
matmul inputs and DMA staging of h.
"""

import numpy as np
import ml_dtypes

# Model dims (fixed by the problem)
B, T, C, H, W = 16, 8, 16, 64, 64
HID, LAT, KS = 32, 64, 3
NCORES = 8
BL = B // NCORES           # images per core

bf16 = ml_dtypes.bfloat16


# ----------------------------------------------------------------------------
# Host-side preprocessing
# ----------------------------------------------------------------------------

def _note_encoder_pe(midi_notes, enc_w1, enc_b1, enc_w2, enc_b2):
    # matches reference: tanh(relu((-0.5 + m/64) @ w1 + b1) @ w2 + b2)
    notes = (-0.5 + midi_notes / np.float32(64.0)).astype(np.float32)
    a = np.maximum(notes @ enc_w1 + enc_b1, 0.0).astype(np.float32)
    pe = np.tanh(a @ enc_w2 + enc_b2).astype(np.float32)
    return pe  # [B, T, C*H*W]


def _paired_weights(w, row_perm, kpair):
    # stack taps (dy,0) and (dy,2): rows [0:cin]=W(dy,0), pad, [kpair-cin:]=W(dy,2)
    cout, cin = w.shape[0], w.shape[1]
    wt = w.transpose(1, 2, 3, 0)          # [cin, 3, 3, cout]
    if row_perm is not None:
        wt = wt[row_perm]
    out = np.zeros((kpair, 3 * cout), np.float32)
    for dy in range(3):
        out[0:cin, dy * cout:(dy + 1) * cout] = wt[:, dy, 0, :]
        out[kpair - cin:, dy * cout:(dy + 1) * cout] = wt[:, dy, 2, :]
    return out.astype(bf16)


def _weights_to_sb(w, row_perm=None):
    # w: [Cout, Cin, 3, 3] -> [Cin, 9*Cout], col = (dy*3+dx)*Cout + co
    cout, cin = w.shape[0], w.shape[1]
    ws = w.transpose(1, 2, 3, 0).reshape(cin, 9 * cout)
    if row_perm is not None:
        ws = ws[row_perm]
    return np.ascontiguousarray(ws).astype(bf16)


# ----------------------------------------------------------------------------
# Device program
# ----------------------------------------------------------------------------

_PROGRAM_CACHE = {}


def _build_program(bl=BL, t_steps=T, hh=H, ww=W, repeats=1):
    """Emit the per-core Bass/Tile program.  Returns nc."""
    import concourse.bass as bass
    import concourse.tile as tile
    from concourse import bacc, mybir

    f32 = mybir.dt.float32
    b16 = mybir.dt.bfloat16
    AF = mybir.ActivationFunctionType

    hp = hh + 2
    padn = hp * hp
    hw = hh * ww
    FOLD = 4
    fw = hw // FOLD                 # free size of a folded tile
    rows_per_block = hh // FOLD     # spatial rows per fold block
    NCH = 8                         # matmul free-dim chunks (<=512 each)
    chunk = hw // NCH
    rows_per_chunk = hh // NCH
    half_hw = hw // 2

    nc = bacc.Bacc("TRN2", target_bir_lowering=False, debug=False, num_devices=1)

    xa = nc.dram_tensor("xa", [bl, t_steps, C, hw], b16, kind="ExternalInput")
    w0f = nc.dram_tensor("w0f", [C + HID, 9 * 4 * HID], b16, kind="ExternalInput")
    w0b = nc.dram_tensor("w0b", [C + HID, 9 * 4 * HID], b16, kind="ExternalInput")
    w1f = nc.dram_tensor("w1f", [3 * HID, 9 * 4 * HID], b16, kind="ExternalInput")
    w1b = nc.dram_tensor("w1b", [3 * HID, 9 * 4 * HID], b16, kind="ExternalInput")
    wfc = nc.dram_tensor("wfc", [2 * HID, 9 * LAT], b16, kind="ExternalInput")
    w0fp = nc.dram_tensor("w0fp", [112, 3 * 4 * HID], b16, kind="ExternalInput")
    w0bp = nc.dram_tensor("w0bp", [112, 3 * 4 * HID], b16, kind="ExternalInput")
    wfcp = nc.dram_tensor("wfcp", [128, 3 * LAT], b16, kind="ExternalInput")
    biases = nc.dram_tensor("biases", [128, 5], f32, kind="ExternalInput")
    out = nc.dram_tensor("out", [bl, LAT, hw], f32, kind="ExternalOutput")
    h0d = nc.dram_tensor("h0d", [bl, t_steps, 2 * HID, hw], b16)

    with tile.TileContext(nc) as tc:
        import contextlib
        with contextlib.ExitStack() as ctx:
            persist = ctx.enter_context(tc.tile_pool(name="persist", bufs=1))
            gates_pool = ctx.enter_context(tc.tile_pool(name="gates", bufs=2))
            psum_pool = ctx.enter_context(
                tc.tile_pool(name="ps", bufs=2, space="PSUM")
            )
            hout_pool = ctx.enter_context(tc.tile_pool(name="hout", bufs=2))

            # ---- persistent tiles -------------------------------------------
            w0f_sb = persist.tile([C + HID, 9 * 4 * HID], b16, tag="w0f")
            w0b_sb = persist.tile([C + HID, 9 * 4 * HID], b16, tag="w0b")
            w1f_sb = persist.tile([3 * HID, 9 * 4 * HID], b16, tag="w1f")
            w1b_sb = persist.tile([3 * HID, 9 * 4 * HID], b16, tag="w1b")
            wfc_sb = persist.tile([2 * HID, 9 * LAT], b16, tag="wfc")
            w0fp_sb = persist.tile([112, 3 * 4 * HID], b16, tag="w0fp")
            w0bp_sb = persist.tile([112, 3 * 4 * HID], b16, tag="w0bp")
            wfcp_sb = persist.tile([128, 3 * LAT], b16, tag="wfcp")
            bias_sb = persist.tile([128, 5], f32, tag="bias")
            nc.sync.dma_start(w0f_sb[:], w0f[:])
            nc.sync.dma_start(w0b_sb[:], w0b[:])
            nc.sync.dma_start(w1f_sb[:], w1f[:])
            nc.sync.dma_start(w1b_sb[:], w1b[:])
            nc.sync.dma_start(wfc_sb[:], wfc[:])
            nc.sync.dma_start(w0fp_sb[:], w0fp[:])
            nc.sync.dma_start(w0bp_sb[:], w0bp[:])
            nc.sync.dma_start(wfcp_sb[:], wfcp[:])
            nc.sync.dma_start(bias_sb[:], biases[:])

            # conv input buffers (zero-padded), one per image+layer
            l0buf = [persist.tile([112, padn], b16, tag=f"l0b{i}",
                                  name=f"l0b{i}") for i in range(bl)]
            l1buf = [persist.tile([3 * HID, padn], b16, tag=f"l1b{i}",
                                  name=f"l1b{i}") for i in range(bl)]
            fcbuf = [persist.tile([128, padn], b16, tag=f"fcb{i}",
                                  name=f"fcb{i}") for i in range(bl)]
            cstate = [persist.tile([128, fw], f32, tag=f"c{i}", name=f"c{i}")
                      for i in range(bl)]

            for i in range(bl):
                nc.vector.memset(l0buf[i][:], 0.0)
                nc.vector.memset(l1buf[i][:], 0.0)
                nc.vector.memset(fcbuf[i][:], 0.0)

            # ---- helpers ----------------------------------------------------
            _dma_rr = [0]

            def dma(dst, src):
                eng = nc.sync if _dma_rr[0] % 2 == 0 else nc.gpsimd
                _dma_rr[0] += 1
                eng.dma_start(dst, src)

            def rhs_slice(buf, cin, tap, j):
                dy, dx = tap // 3, tap % 3
                r0 = rows_per_chunk * j + dy
                return (buf[0:cin, :]
                        .rearrange("p (r c) -> p r c", c=hp)
                        [:, r0:r0 + rows_per_chunk, dx:dx + ww])

            def interior(buf, pbase, pcnt, a, shift=0):
                r0, nr = rows_per_block * a, rows_per_block
                off = (r0 + 1) * hp + 1 - shift
                return (buf[pbase:pbase + pcnt, off:off + nr * hp]
                        .rearrange("p (r c) -> p r c", c=hp)[:, :, 0:ww])

            def conv_z_to_S(buf, cin, w_sb, bias_col, S, mparts=128,
                            act=None, Fg=None, pair=None):
                """z = conv(buf) for all output channels; nonlinearity applied
                while evacuating PSUM into S [mparts, hw]."""
                for hlf in range(2):
                    ps = psum_pool.tile([128, half_hw], f32, tag="ps",
                                        name="ps")
                    for jj in range(NCH // 2):
                        j = hlf * (NCH // 2) + jj
                        if pair is None:
                            for tap in range(9):
                                nc.tensor.matmul(
                                    ps[0:mparts,
                                       chunk * jj:chunk * (jj + 1)],
                                    w_sb[0:cin,
                                         tap * mparts:(tap + 1) * mparts],
                                    rhs_slice(buf, cin, tap, j),
                                    start=(tap == 0), stop=(tap == 8),
                                )
                        else:
                            wp_sb, kpair = pair
                            for sweep in range(6):
                                dy = sweep % 3
                                if sweep < 3:   # paired (dy,0)+(dy,2)
                                    lhsT = wp_sb[0:kpair,
                                                 dy * mparts:(dy + 1) * mparts]
                                    rhs = rhs_slice(buf, kpair, 3 * dy, j)
                                else:           # single (dy,1)
                                    lhsT = w_sb[0:cin,
                                                (3 * dy + 1) * mparts:
                                                (3 * dy + 2) * mparts]
                                    rhs = rhs_slice(buf, cin, 3 * dy + 1, j)
                                nc.tensor.matmul(
                                    ps[0:mparts,
                                       chunk * jj:chunk * (jj + 1)],
                                    lhsT, rhs,
                                    start=(sweep == 0), stop=(sweep == 5),
                                )
                    cols = slice(half_hw * hlf, half_hw * (hlf + 1))
                    if act == "gates":
                        nc.scalar.activation(
                            S[0:96, cols], ps[0:96, :], AF.Sigmoid,
                            bias=bias_sb[0:96, bias_col:bias_col + 1])
                        for jj in range(NCH // 2):
                            j = hlf * (NCH // 2) + jj
                            a, wh = j // 2, j % 2
                            nc.scalar.activation(
                                Fg[32 * a:32 * a + 32,
                                   chunk * wh:chunk * (wh + 1)],
                                ps[96:128, chunk * jj:chunk * (jj + 1)],
                                AF.Tanh,
                                bias=bias_sb[96:128, bias_col:bias_col + 1])
                    else:
                        nc.scalar.activation(
                            S[0:mparts, cols], ps[0:mparts, :], AF.Identity,
                            bias=bias_sb[0:mparts, bias_col:bias_col + 1])

            def fold(S, q, name):
                Ft = gates_pool.tile([128, fw], f32, tag=name, name=name)
                for a in range(FOLD):
                    dma(Ft[32 * a:32 * a + 32, :],
                        S[32 * q:32 * q + 32, fw * a:fw * (a + 1)])
                return Ft

            def lstm_step(img, buf, cin, w_sb, bias_col, first,
                          h_targets, h0_store, pair=None):
                """One ConvLSTM cell step (gate channel order i,f,o,g)."""
                c = cstate[img]
                S = gates_pool.tile([96, hw], f32, tag="S", name="S")
                Fg = gates_pool.tile([128, fw], f32, tag="Fg", name="Fg")
                conv_z_to_S(buf, cin, w_sb, bias_col, S, act="gates", Fg=Fg,
                            pair=pair)

                Fi = fold(S, 0, "Fi")
                if first:
                    # c = sigmoid(i) * tanh(g)
                    nc.vector.tensor_mul(c[:], Fi[:], Fg[:])
                else:
                    Ff = fold(S, 1, "Ff")
                    t1 = gates_pool.tile([128, fw], f32, tag="t1", name="t1")
                    nc.vector.tensor_mul(t1[:], Fi[:], Fg[:])
                    t2 = gates_pool.tile([128, fw], f32, tag="t2", name="t2")
                    nc.vector.tensor_mul(t2[:], Ff[:], c[:])
                    nc.vector.tensor_add(c[:], t1[:], t2[:])
                Fo = fold(S, 2, "Fo")

                tc_sb = gates_pool.tile([128, fw], f32, tag="tc_sb",
                                        name="tc_sb")
                nc.scalar.activation(tc_sb[:], c[:], AF.Tanh)

                h2 = hout_pool.tile([128, fw], b16, tag="h2", name="h2")
                nc.vector.tensor_mul(h2[:], Fo[:], tc_sb[:])

                for tbuf, pbase, shifted in h_targets:
                    for a in range(FOLD):
                        dma(interior(tbuf, pbase, 32, a),
                            h2[32 * a:32 * a + 32, :])
                        if shifted is not None:
                            dma(interior(tbuf, shifted, 32, a, shift=2),
                                h2[32 * a:32 * a + 32, :])
                if h0_store is not None:
                    for a in range(FOLD):
                        dma(h0_store[:, fw * a:fw * (a + 1)],
                            h2[32 * a:32 * a + 32, :])

            def load_x_l0(img, t, buf):
                # xa[img, t] -> x region (parts HID:HID+C) interior,
                # plus the +2-shifted copy at parts 96:112
                half = hh // 2
                for r in range(2):
                    src = xa[img, t, :, r * half * ww:(r + 1) * half * ww]
                    off = (r * half + 1) * hp + 1
                    dst = (buf[HID:HID + C, off:off + half * hp]
                           .rearrange("p (r c) -> p r c", c=hp)[:, :, 0:ww])
                    dma(dst, src)
                    dst2 = (buf[96:96 + C, off - 2:off - 2 + half * hp]
                            .rearrange("p (r c) -> p r c", c=hp)[:, :, 0:ww])
                    dma(dst2, src)

            def load_x_l1(img, t, buf):
                # h0d[img, t] (64 ch) -> parts 0:64 interior
                half = hh // 2
                for d in range(2):
                    for r in range(2):
                        src = h0d[img, t, d * HID:(d + 1) * HID,
                                  r * half * ww:(r + 1) * half * ww]
                        off = (r * half + 1) * hp + 1
                        dst = (buf[d * HID:(d + 1) * HID, off:off + half * hp]
                               .rearrange("p (r c) -> p r c", c=hp)
                               [:, :, 0:ww])
                        dma(dst, src)

            # ---- the schedule ----------------------------------------------
            for _rep in range(repeats):
                # layer 0, forward then backward
                for direction in ("f", "b"):
                    w_sb = {"f": w0f_sb, "b": w0b_sb}[direction]
                    bias_col = {"f": 0, "b": 1}[direction]
                    dirb = {"f": 0, "b": 1}[direction]
                    wp_sb = {"f": w0fp_sb, "b": w0bp_sb}[direction]
                    for img in range(bl):
                        nc.vector.memset(cstate[img][:], 0.0)
                        nc.vector.memset(l0buf[img][0:HID, :], 0.0)
                        nc.vector.memset(l0buf[img][64:64 + HID, :], 0.0)
                        load_x_l0(img, t_steps - 1 if direction == "b" else 0,
                                  l0buf[img])
                    for k in range(t_steps):
                        t = (t_steps - 1 - k) if direction == "b" else k
                        t_nxt = (t - 1) if direction == "b" else (t + 1)
                        for img in range(bl):
                            lstm_step(
                                img, l0buf[img], C + HID, w_sb, bias_col,
                                first=(k == 0),
                                h_targets=([(l0buf[img], 0, 64)]
                                           if k + 1 < t_steps else []),
                                h0_store=h0d[img, t,
                                             dirb * HID:(dirb + 1) * HID, :],
                                pair=(wp_sb, 112),
                            )
                            if k + 1 < t_steps:
                                load_x_l0(img, t_nxt, l0buf[img])

                # layer 1 forward
                for img in range(bl):
                    nc.vector.memset(cstate[img][:], 0.0)
                    nc.vector.memset(l1buf[img][2 * HID:3 * HID, :], 0.0)
                    load_x_l1(img, 0, l1buf[img])
                for k in range(t_steps):
                    for img in range(bl):
                        last = (k == t_steps - 1)
                        lstm_step(
                            img, l1buf[img], 3 * HID, w1f_sb, 2,
                            first=(k == 0),
                            h_targets=([(fcbuf[img], 0, 64)] if last
                                       else [(l1buf[img], 2 * HID, None)]),
                            h0_store=None,
                        )
                        if not last:
                            load_x_l1(img, k + 1, l1buf[img])

                # layer 1 backward: single step on x = h0[T-1], h = c = 0
                for img in range(bl):
                    nc.vector.memset(cstate[img][:], 0.0)
                    nc.vector.memset(l1buf[img][2 * HID:3 * HID, :], 0.0)
                    load_x_l1(img, t_steps - 1, l1buf[img])
                    lstm_step(
                        img, l1buf[img], 3 * HID, w1b_sb, 3, first=True,
                        h_targets=[(fcbuf[img], HID, 96)], h0_store=None,
                    )

                # final conv: fcbuf (64ch) -> out (64ch) + bias
                for img in range(bl):
                    Z = gates_pool.tile([64, hw], f32, tag="Z", name="Z")
                    conv_z_to_S(fcbuf[img], 2 * HID, wfc_sb, 4, Z,
                                mparts=64, act=None, pair=(wfcp_sb, 128))
                    for hlf in range(2):
                        nc.sync.dma_start(
                            out[img, :, half_hw * hlf:half_hw * (hlf + 1)],
                            Z[0:64, half_hw * hlf:half_hw * (hlf + 1)])

    nc.compile()
    return nc


def _get_program(key=(BL, T, H, W)):
    if key not in _PROGRAM_CACHE:
        _PROGRAM_CACHE[key] = _build_program(*key)
    return _PROGRAM_CACHE[key]


# ----------------------------------------------------------------------------
# Entry point
# ----------------------------------------------------------------------------

def _prep_in_maps(x_audio, midi_notes, enc_w1, enc_b1, enc_w2, enc_b2,
                  w0f, b0f, w0b, b0b, w1f, b1f, w1b, b1b, fc_w, fc_b,
                  n_cores=NCORES):
    bsz = x_audio.shape[0]
    bl = bsz // n_cores
    pe = _note_encoder_pe(midi_notes, enc_w1, enc_b1, enc_w2, enc_b2)
    x = (np.asarray(x_audio, np.float32)
         + pe.reshape(x_audio.shape)).astype(bf16)
    x = x.reshape(bsz, T, C, H * W)

    perm0 = list(range(C, C + HID)) + list(range(C))
    w0f_t, w0b_t = _weights_to_sb(w0f, perm0), _weights_to_sb(w0b, perm0)
    w1f_t, w1b_t = _weights_to_sb(w1f), _weights_to_sb(w1b)
    wfc_t = _weights_to_sb(fc_w)

    w0fp_t = _paired_weights(w0f, perm0, 112)
    w0bp_t = _paired_weights(w0b, perm0, 112)
    wfcp_t = _paired_weights(fc_w, None, 128)

    bias_cols = []
    for b in (b0f, b0b, b1f, b1b):
        bias_cols.append(np.asarray(b, np.float32))
    bias_cols.append(np.pad(np.asarray(fc_b, np.float32), (0, 64)))
    biases = np.stack(bias_cols, axis=1).astype(np.float32)  # [128, 5]

    in_maps = []
    for core in range(n_cores):
        sl = slice(core * bl, (core + 1) * bl)
        in_maps.append({
            "xa": np.ascontiguousarray(x[sl]),
            "w0f": w0f_t, "w0b": w0b_t, "w1f": w1f_t, "w1b": w1b_t,
            "wfc": wfc_t, "w0fp": w0fp_t, "w0bp": w0bp_t, "wfcp": wfcp_t,
            "biases": biases,
        })
    return in_maps


def kernel(x_audio, midi_notes, enc_w1, enc_b1, enc_w2, enc_b2,
           w0f, b0f, w0b, b0b, w1f, b1f, w1b, b1b, fc_w, fc_b):
    from concourse.bass_utils import run_bass_kernel_spmd

    nc = _get_program()
    in_maps = _prep_in_maps(
        x_audio, midi_notes, enc_w1, enc_b1, enc_w2, enc_b2,
        w0f, b0f, w0b, b0b, w1f, b1f, w1b, b1b, fc_w, fc_b,
    )
    res = run_bass_kernel_spmd(nc, in_maps, list(range(NCORES)))
    outs = [r["out"].reshape(BL, LAT, H, W) for r in res.results]
    return np.concatenate(outs, axis=0).astype(np.float32)

